# revision 6
# baseline (speedup 1.0000x reference)
"""Conv2dfft forward on 8 TRN2 NeuronCores.

The reference computes cross-correlation via rfft2/irfft2 on a 65x65 grid.
Because the FFT grid (65) >= padded_H + KH - 1 (34 + 3 - 1 = 36 would wrap,
but the output is cropped to out_H=32 and max input row touched is 33 < 65),
no circular wraparound reaches the cropped output: the result is EXACTLY a
3x3 same-padding cross-correlation (DL conv, padding=1) plus bias.

So: direct conv as 9 PSUM-accumulated matmuls per output tile.
  out[f, (y,x)] = sum_t sum_c wT[c, t, f] * xpad[c, y+dy_t, x+dx_t]
with contraction over C=128 on the partition dim.

Sharding: data-parallel over batch N=32 -> 4 images per core.
"""

import os
from contextlib import ExitStack

import numpy as np

import concourse.bacc as bacc
import concourse.mybir as mybir
import concourse.tile as tile
from concourse import bass_utils

N_CORES = 8
N, C, H, W = 32, 128, 32, 32
F = 128
KH = KW = 3
PH, PW = H + 2, W + 2          # padded input 34x34
NLOC = N // N_CORES            # images per core
CH = 16                        # output rows per chunk (CH*W = 512 = one PSUM bank)
NCHUNK = H // CH

# matmul input dtype: "f32r" (full-rate fp32-stored), "f32" (4x slower, exact),
# "bf16" (full rate, half DMA, lower precision)
MM_DT = os.environ.get("CONV_MM_DT", "f32r")

# Number of times the compute body is emitted in the NEFF (timing tool:
# per-iteration HW time = (t_K - t_1) / (K - 1), host-RPC overhead cancels).
N_ITERS = int(os.environ.get("CONV_ITERS", "1"))

_DT_MAP = {
    "f32r": mybir.dt.float32r,
    "f32": mybir.dt.float32,
    "bf16": mybir.dt.bfloat16,
}

_cached_nc = None
LAST_RESULT = None


def _build():
    dt_mm = _DT_MAP[MM_DT]
    nc = bacc.Bacc(
        "TRN2",
        target_bir_lowering=False,
        debug=False,
        num_devices=N_CORES,
    )

    xp = nc.dram_tensor("xp", [NLOC, C, PH, PW], dt_mm, kind="ExternalInput")
    wt = nc.dram_tensor("wt", [C, KH * KW * F], dt_mm, kind="ExternalInput")
    bb = nc.dram_tensor("bb", [F, 1], mybir.dt.float32, kind="ExternalInput")
    out = nc.dram_tensor("out", [NLOC, F, H, W], mybir.dt.float32, kind="ExternalOutput")

    xp_ap = xp.ap()
    out_ap = out.ap()

    with ExitStack() as ctx:
        tc = ctx.enter_context(tile.TileContext(nc))
        const_pool = ctx.enter_context(tc.tile_pool(name="const", bufs=1))
        x_pool = ctx.enter_context(tc.tile_pool(name="xs", bufs=3))
        ps_pool = ctx.enter_context(tc.tile_pool(name="ps", bufs=8, space="PSUM"))
        o_pool = ctx.enter_context(tc.tile_pool(name="os", bufs=4))

        wt_sb = const_pool.tile([C, KH * KW * F], dt_mm)
        nc.sync.dma_start(wt_sb[:], wt.ap())
        b_sb = const_pool.tile([F, 1], mybir.dt.float32)
        nc.sync.dma_start(b_sb[:], bb.ap())

        for _ in range(N_ITERS):
            for n in range(NLOC):
                x_sb = x_pool.tile([C, PH, PW], dt_mm)
                nc.sync.dma_start(x_sb[:], xp_ap[n, :, :, :])
                for h in range(NCHUNK):
                    ps = ps_pool.tile([F, CH, W], mybir.dt.float32)
                    t = 0
                    for dy in range(KH):
                        for dx in range(KW):
                            rhs = x_sb[:, h * CH + dy : h * CH + dy + CH, dx : dx + W]
                            nc.tensor.matmul(
                                ps[:],
                                wt_sb[:, t * F : (t + 1) * F],
                                rhs,
                                start=(t == 0),
                                stop=(t == KH * KW - 1),
                            )
                            t += 1
                    o_sb = o_pool.tile([F, CH, W], mybir.dt.float32)
                    nc.vector.tensor_scalar_add(o_sb[:], ps[:], b_sb[:])
                    nc.sync.dma_start(out_ap[n, :, h * CH : (h + 1) * CH, :], o_sb[:])

    nc.compile()
    return nc


def _np_mm_dtype():
    if MM_DT == "bf16":
        import ml_dtypes

        return np.dtype(ml_dtypes.bfloat16)
    return np.dtype(np.float32)


def kernel(x: np.ndarray, w: np.ndarray, b: np.ndarray) -> np.ndarray:
    global _cached_nc, LAST_RESULT
    if _cached_nc is None:
        _cached_nc = _build()
    nc = _cached_nc

    np_dt = _np_mm_dtype()
    x = np.asarray(x, dtype=np.float32)
    w = np.asarray(w, dtype=np.float32)
    b = np.asarray(b, dtype=np.float32)

    xp = np.zeros((N, C, PH, PW), dtype=np_dt)
    xp[:, :, 1 : 1 + H, 1 : 1 + W] = x
    # wt[c, (dy*KW+dx)*F + f] = w[f, c, dy, dx]
    wt = np.ascontiguousarray(w.transpose(1, 2, 3, 0)).reshape(C, KH * KW * F)
    wt = wt.astype(np_dt)
    bb = np.ascontiguousarray(b.reshape(F, 1))

    in_maps = [
        {"xp": xp[i * NLOC : (i + 1) * NLOC], "wt": wt, "bb": bb}
        for i in range(N_CORES)
    ]
    res = bass_utils.run_bass_kernel_spmd(
        nc,
        in_maps,
        list(range(N_CORES)),
        trace=bool(int(os.environ.get("CONV_TRACE", "0"))),
    )
    LAST_RESULT = res
    return np.concatenate([r["out"] for r in res.results], axis=0)
